# revision 26
# baseline (speedup 1.0000x reference)
"""Trainium2 Bass kernel v4 for quantized-MLP-with-LoRA.

Data-parallel over tokens (1024/core). All host-side prep is layout/dtype
only (transpose, tiling, exact integer remap 2q-15, exact scale halving,
f16 casts — the same casts the device DMA performed in v3).

v4 over v3:
- x1 arrives pre-transposed/cast as [P, KD, T] f16: no on-device transposes,
  ~3x smaller startup DMA, loaded in parallel on two idle queues.
- codes arrive as int8 (2q-15), cast to f16 during the SWDGE DMA: 4x less
  HBM read traffic for codes, and dequant collapses to ONE DVE mult by the
  host-halved scales (w = (2q-15) * (s/2)).
- biases/LoRA constants arrive pre-tiled in f16: no staging copies.
- down phase is chunk-granular (16 kh-tiles per chunk) and shares the up
  phase's staging pools, so the first down chunks load+dequant during the
  up-phase tail: no PE stall at the phase transition.
- y2 is stored as f16 (host up-casts while gathering): half the store DMA.
"""
import sys

if "/opt/trn_rl_repo" not in sys.path:
    sys.path.insert(0, "/opt/trn_rl_repo")

import ml_dtypes
import numpy as np

import concourse.bass as bass
import concourse.mybir as mybir
import concourse.tile as tile
from concourse import bacc
from concourse.bass import ts, ds
from concourse.bass_utils import run_bass_kernel_spmd

F16 = mybir.dt.float16
F32 = mybir.dt.float32
F8 = mybir.dt.float8e4
I8 = mybir.dt.int8

NCORES = 8
T = 1024          # tokens per core
D = 2048
H = 8192
R = 16
P = 128
KD = D // P       # 16 contraction tiles for up
KH = H // P       # 64 contraction tiles for down
DT = D // P       # 16 output d-tiles for down
NT = T // 512     # 2 moving-operand chunks of 512 tokens
NC = 4            # down-phase chunks per d-tile (16 kh-tiles each)
KC = KH // NC     # kh-tiles per down chunk

TRACE = False
LAST_RESULTS = None
_NC = None


def _build():
    nc = bacc.Bacc("TRN2", target_bir_lowering=False, debug=False,
                   enable_asserts=False, num_devices=NCORES)

    x1td = nc.dram_tensor("x1td", [P, KD, T], F16, kind="ExternalInput").ap()
    wupL = nc.dram_tensor("wupL", [KH, P, D], I8, kind="ExternalInput").ap()
    supL = nc.dram_tensor("supL", [KH, P, D], F16, kind="ExternalInput").ap()
    bupv = nc.dram_tensor("bupv", [P, KH], F32, kind="ExternalInput").ap()
    a1f_d = nc.dram_tensor("a1f", [P, KD, R], F16, kind="ExternalInput").ap()
    b1f_d = nc.dram_tensor("b1f", [R, H], F16, kind="ExternalInput").ap()
    wdnL = nc.dram_tensor("wdnL", [DT, P, H], I8, kind="ExternalInput").ap()
    sdnL = nc.dram_tensor("sdnL", [DT, P, H], F16, kind="ExternalInput").ap()
    a2f_d = nc.dram_tensor("a2f", [P, KH, R], F8, kind="ExternalInput").ap()
    b2p_d = nc.dram_tensor("b2p", [R + 1, D], F16, kind="ExternalInput").ap()
    y2t = nc.dram_tensor("y2t", [D, T], F16, kind="ExternalOutput").ap()

    with tile.TileContext(nc) as tc:
        with tc.tile_pool(name="big", bufs=1) as bp, \
             tc.tile_pool(name="const", bufs=1) as cp, \
             tc.tile_pool(name="qstage", bufs=4) as qp, \
             tc.tile_pool(name="sstage", bufs=3) as sfp, \
             tc.tile_pool(name="b1stage", bufs=4) as b1p, \
             tc.tile_pool(name="x8stage", bufs=3) as x8p, \
             tc.tile_pool(name="yout", bufs=3) as yp, \
             tc.tile_pool(name="psum", bufs=6, space="PSUM") as pp, \
             tc.tile_pool(name="psum_vt", bufs=1, space="PSUM") as pvt:

            # resident hidden activation: x2sb[h_part, kh, tok]
            x2sb = bp.tile([P, KH, T], F16, tag="x2sb")
            x1t = cp.tile([P, KD, T], F16, tag="x1t")
            # a2 arrives pre-scaled by 64 in fp8 (b2 is pre-divided by 64):
            # the vt lora matmuls run in fp8 DoubleRow, pairing k-slabs.
            # the lora term is ~2% of y2, so fp8's ~2% error is ~4e-4 of y2.
            a2f = cp.tile([P, KH, R], F8, tag="a2f")
            a1f = cp.tile([P, KD, R], F16, tag="a1f")
            b2p = cp.tile([R + 1, D], F16, tag="b2p")
            bupsb = cp.tile([P, KH], F32, tag="bupsb")
            utf = cp.tile([R, T], F16, tag="utf")
            v1t = cp.tile([R + 1, T], F16, tag="v1t")
            # row R of v1t stays 1.0 -> folds b_down into the lora matmul
            nc.any.memset(v1t[:], 1.0)

            vt_ps = [pvt.tile([R, 512], F32, tag=f"vt{i}", name=f"vt{i}")
                     for i in range(NT)]

            # ---- staging pipeline (shared by up slabs and down chunks) ----
            def up_load(k):
                qf = qp.tile([P, D], F16, tag="qf", name=f"uq{k}")
                nc.gpsimd.dma_start(qf[:], wupL[k])          # i8 -> f16 cast
                sfl = sfp.tile([P, D], F16, tag="sfl", name=f"us{k}")
                nc.sync.dma_start(sfl[:], supL[k])
                b1s = b1p.tile([R, P], F16, tag="b1s", name=f"b1_{k}")
                nc.sync.dma_start(b1s[:], b1f_d[:, ts(k, P)])
                return qf, sfl, b1s

            def dn_load(q):
                dt, c = q // NC, q % NC
                qf = qp.tile([P, D], F16, tag="qf", name=f"dq{q}")
                nc.gpsimd.dma_start(qf[:], wdnL[dt, :, ts(c, D)])
                sfl = sfp.tile([P, D], F16, tag="sfl", name=f"ds{q}")
                nc.sync.dma_start(sfl[:], sdnL[dt, :, ts(c, D)])
                return qf, sfl, None

            def dequant(loaded):
                qf, sfl, b1s = loaded
                nc.vector.tensor_tensor(qf[:], qf[:], sfl[:],
                                        mybir.AluOpType.mult)
                return qf, b1s

            # ================= UP phase =================
            # software pipeline: loads k+2, dequant k+1, matmuls k.
            # the first two DOWN chunks load/dequant during the up tail.
            # k=0 slab arrives in quarters so the first ldweights issues at
            # ~2.6us instead of waiting for the whole 1MB slab + dequant
            def up_load0():
                qf = qp.tile([P, D], F16, tag="qf", name="uq0")
                sfl = sfp.tile([P, D], F16, tag="sfl", name="us0")
                for h in range(4):
                    nc.gpsimd.dma_start(qf[:, ts(h, 512)], wupL[0, :, ts(h, 512)])
                    nc.sync.dma_start(sfl[:, ts(h, 512)], supL[0, :, ts(h, 512)])
                b1s = b1p.tile([R, P], F16, tag="b1s", name="b1_0")
                nc.sync.dma_start(b1s[:], b1f_d[:, ts(0, P)])
                return qf, sfl, b1s

            def dequant0(loaded):
                qf, sfl, b1s = loaded
                for h in range(4):
                    nc.vector.tensor_tensor(qf[:, ts(h, 512)], qf[:, ts(h, 512)],
                                            sfl[:, ts(h, 512)],
                                            mybir.AluOpType.mult)
                return qf, b1s

            LD, DQ, x28p = {}, {}, {}
            LD[0] = up_load0()

            # x1t arrives in 4 j-major pieces, alternating HWDGE queues, so
            # the k=0 slab's j-matmuls start while later pieces stream in.
            # the earliest-needed piece rides the empty Act queue; only the
            # k=0 weight slab precedes the pieces (slab 1 isn't needed until
            # ~24us, so it loads after x1t to keep the startup pipe clear).
            nc.scalar.dma_start(x1t[:, 0:4, :], x1td[:, 0:4, :])
            nc.sync.dma_start(x1t[:, 4:8, :], x1td[:, 4:8, :])
            nc.scalar.dma_start(x1t[:, 8:12, :], x1td[:, 8:12, :])
            nc.sync.dma_start(x1t[:, 12:16, :], x1td[:, 12:16, :])
            nc.sync.dma_start(a1f[:], a1f_d)
            LD[1] = up_load(1)
            nc.sync.dma_start(bupsb[:], bupv)
            nc.sync.dma_start(a2f[:], a2f_d)
            nc.sync.dma_start(b2p[:], b2p_d)
            DQ[0] = dequant0(LD.pop(0))

            for k in range(KH):
                if k + 2 < KH:
                    LD[k + 2] = up_load(k + 2)
                else:
                    LD[k + 2] = dn_load(k + 2 - KH)
                DQ[k + 1] = dequant(LD.pop(k + 1))
                qf, b1s = DQ.pop(k)
                wt = qf.rearrange("p (j h) -> p j h", h=P)

                psk = [pp.tile([P, 512], F32, tag="mm", name=f"up{k}_{tt}")
                       for tt in range(NT)]
                if k % 2 == 0:
                    x28 = x8p.tile([P, 2, T], F8, tag="x28", name=f"x28_{k//2}")
                    x28p[k // 2] = x28
                for j in range(KD):
                    for tt in range(NT):
                        nc.tensor.matmul(psk[tt][:], wt[:, j, :],
                                         x1t[:, j, ts(tt, 512)],
                                         start=(j == 0), stop=False)
                    # deferred vt pair (k-3, k-2): fp8 DoubleRow contracts two
                    # k-slabs per instruction; the 2-slab deferral gives the
                    # Act relu and the DVE fp8 copy time to land
                    if k >= 3 and k % 2 == 1 and j in (2, 4):
                        m = (k - 3) // 2
                        tt8 = 0 if j == 2 else 1
                        nc.tensor.matmul(vt_ps[tt8][:],
                                         a2f[:, 2 * m:2 * m + 2, :],
                                         x28p[m][:, :, ts(tt8, 512)],
                                         start=(m == 0), stop=False,
                                         perf_mode=mybir.MatmulPerfMode.DoubleRow,
                                         skip_group_check=True)
                        if tt8 == 1:
                            x28p.pop(m)
                if k == 0:
                    # uT = (x1 @ A1)^T : [R, T] — runs here, after the k=0
                    # j-loop (x1t fully arrived), before the b1s close that
                    # consumes utf. The open psk chains sit in other banks.
                    for tt in range(NT):
                        ups = pp.tile([R, 512], F32, tag="mm", name=f"uT{tt}")
                        for j in range(KD):
                            nc.tensor.matmul(ups[:], a1f[:, j, :],
                                             x1t[:, j, ts(tt, 512)],
                                             start=(j == 0), stop=(j == KD - 1),
                                             skip_group_check=True)
                        nc.scalar.copy(utf[:, ts(tt, 512)], ups[:])
                for tt in range(NT):
                    nc.tensor.matmul(psk[tt][:], b1s[:], utf[:, ts(tt, 512)],
                                     start=False, stop=True)
                    nc.scalar.activation(x2sb[:, k, ts(tt, 512)], psk[tt][:],
                                         mybir.ActivationFunctionType.Relu,
                                         bias=bupsb[:, k:k + 1], scale=1.0)
                nc.vector.tensor_copy(x28p[k // 2][:, k % 2, :], x2sb[:, k, :])
            # tail vt pair 31 (pair 30 was consumed in-loop at k=63)
            m = KH // 2 - 1
            for tt in range(NT):
                nc.tensor.matmul(vt_ps[tt][:], a2f[:, 2 * m:2 * m + 2, :],
                                 x28p[m][:, :, ts(tt, 512)],
                                 start=False, stop=True,
                                 perf_mode=mybir.MatmulPerfMode.DoubleRow,
                                 skip_group_check=True)
            for tt in range(NT):
                nc.scalar.copy(v1t[:R, ts(tt, 512)], vt_ps[tt][:])

            # ================= DOWN phase =================
            # chunk-granular: q = dt*NC + c ; chunks 0,1 staged by the up loop
            NQ = DT * NC
            DQ = {0: DQ.pop(KH)}
            LD = {1: LD.pop(KH + 1)}

            dps = {}
            for q in range(NQ):
                dt, c = q // NC, q % NC
                if q + 2 < NQ:
                    LD[q + 2] = dn_load(q + 2)
                if q + 1 < NQ:
                    DQ[q + 1] = dequant(LD.pop(q + 1))
                qf, _ = DQ.pop(q)
                wch = qf.rearrange("p (kk d) -> p kk d", d=P)

                if c == 0:
                    dps = {tt: pp.tile([P, 512], F32, tag="mm",
                                       name=f"dn{dt}_{tt}")
                           for tt in range(NT)}
                for kk in range(KC):
                    k = c * KC + kk
                    for tt in range(NT):
                        nc.tensor.matmul(dps[tt][:], wch[:, kk, :],
                                         x2sb[:, k, ts(tt, 512)],
                                         start=(k == 0), stop=False)
                if c == NC - 1:
                    for tt in range(NT):
                        nc.tensor.matmul(dps[tt][:], b2p[:, ts(dt, P)],
                                         v1t[:, ts(tt, 512)],
                                         start=False, stop=True)
                        yo = yp.tile([P, 512], F16, tag="yo")
                        nc.scalar.copy(yo[:], dps[tt][:])
                        if dt == DT - 1 and tt == NT - 1:
                            # very last store: halves on both HWDGE queues
                            nc.sync.dma_start(
                                y2t[ts(dt, P), ds(tt * 512, 256)], yo[:, :256])
                            nc.scalar.dma_start(
                                y2t[ts(dt, P), ds(tt * 512 + 256, 256)],
                                yo[:, 256:])
                        else:
                            nc.scalar.dma_start(y2t[ts(dt, P), ts(tt, 512)],
                                                yo[:])

    nc.compile()
    return nc


def build_in_maps(inputs):
    x1 = np.ascontiguousarray(np.asarray(inputs["x1"], dtype=np.float32))
    B, S, _ = x1.shape
    xf = x1.reshape(B * S, D)

    wq = np.asarray(inputs["w_up_q"], dtype=np.int32)         # [H, D]
    sup = np.asarray(inputs["w_up_scale"], dtype=np.float32)  # [H, 32]
    # codes 2q-15 (exact int8), lhsT-tiled:
    # wupL[k][p, j*128+h] = 2*w_up_q[k*128+h, j*128+p] - 15
    wq8 = (2 * wq - 15).astype(np.int8).T                     # [D, H]
    wupL = np.ascontiguousarray(
        wq8.reshape(KD, P, KH, P).transpose(2, 1, 0, 3).reshape(KH, P, D))
    # halved scales expanded to the same layout (replication only, f16)
    sfull = np.repeat(sup.T * 0.5, 64, axis=0)                # [D, H]
    supL = np.ascontiguousarray(
        sfull.reshape(KD, P, KH, P).transpose(2, 1, 0, 3).reshape(KH, P, D)
    ).astype(np.float16)

    wdq = np.asarray(inputs["w_down_q"], dtype=np.int32)      # [D, H]
    sdn = np.asarray(inputs["w_down_scale"], dtype=np.float32)  # [D, 128]
    wdq8 = (2 * wdq - 15).astype(np.int8).T                   # [H, D]
    wdnL = np.ascontiguousarray(
        wdq8.reshape(KH, P, DT, P).transpose(2, 1, 0, 3).reshape(DT, P, H))
    sdfull = np.repeat(sdn.T * 0.5, 64, axis=0)               # [H, D]
    sdnL = np.ascontiguousarray(
        sdfull.reshape(KH, P, DT, P).transpose(2, 1, 0, 3).reshape(DT, P, H)
    ).astype(np.float16)

    a1 = np.asarray(inputs["w_up_lora_a"], dtype=np.float32)   # [D, R]
    a2 = np.asarray(inputs["w_down_lora_a"], dtype=np.float32)  # [H, R]
    b2 = np.asarray(inputs["w_down_lora_b"], dtype=np.float32)  # [R, D]
    bdn = np.asarray(inputs["b_down"], dtype=np.float32)       # [D]
    bup = np.asarray(inputs["b_up"], dtype=np.float32)         # [H]

    shared = {
        "wupL": wupL, "supL": supL,
        "bupv": np.ascontiguousarray(bup.reshape(KH, P).T),
        "a1f": np.ascontiguousarray(
            a1.reshape(KD, P, R).transpose(1, 0, 2)).astype(np.float16),
        "b1f": np.asarray(inputs["w_up_lora_b"], np.float32).astype(np.float16),
        "wdnL": wdnL, "sdnL": sdnL,
        # a2 pre-scaled by 64 (exact pow2) so fp8 e4m3 values stay normal;
        # compensated by b2/64 below. The vt matmuls run in fp8 DoubleRow.
        "a2f": np.ascontiguousarray(
            (a2 * 64.0).reshape(KH, P, R).transpose(1, 0, 2)
        ).astype(ml_dtypes.float8_e4m3),
        "b2p": np.ascontiguousarray(
            np.concatenate([b2 / 64.0, bdn[None, :]], axis=0)
        ).astype(np.float16),
    }
    out = []
    for c in range(NCORES):
        xs = xf[c * T:(c + 1) * T]                             # [T, D]
        x1td = np.ascontiguousarray(
            xs.T.reshape(KD, P, T).transpose(1, 0, 2)).astype(np.float16)
        out.append({"x1td": x1td, **shared})
    return out


def kernel(x1, w_up_q, w_up_scale, b_up, w_up_lora_a, w_up_lora_b,
           w_down_q, w_down_scale, b_down, w_down_lora_a, w_down_lora_b):
    global _NC, LAST_RESULTS
    if _NC is None:
        _NC = _build()

    inputs = dict(x1=x1, w_up_q=w_up_q, w_up_scale=w_up_scale, b_up=b_up,
                  w_up_lora_a=w_up_lora_a, w_up_lora_b=w_up_lora_b,
                  w_down_q=w_down_q, w_down_scale=w_down_scale, b_down=b_down,
                  w_down_lora_a=w_down_lora_a, w_down_lora_b=w_down_lora_b)
    in_maps = build_in_maps(inputs)
    res = run_bass_kernel_spmd(_NC, in_maps, core_ids=list(range(NCORES)),
                               trace=TRACE)
    LAST_RESULTS = res
    B, S, _ = np.asarray(x1).shape
    # y2t is [D, T] f16 per core — un-transpose/up-cast on the host
    out = np.concatenate(
        [res.results[c]["y2t"].T.astype(np.float32) for c in range(NCORES)],
        axis=0)
    return np.ascontiguousarray(out).reshape(B, S, D)
